# revision 14
# baseline (speedup 1.0000x reference)
"""Multi-head attention kernel for 8 Trainium2 NeuronCores.

Problem: B=2, S=2048, D=1024, H=16 heads (Dh=64).
    qh = split(q @ wq.T + bq); kh, vh likewise
    out = concat_h(softmax(qh kh^T / 8) vh) @ wo.T + bo

Sharding: core c = 4*b + j handles batch b and head group j (4 heads,
channels [256j, 256j+256)).  Each core computes its 4 heads' attention and
a partial output projection; the host sums the 4 partials per batch and
adds the constant bv @ wo.T + bo vector.

All matmuls run in bf16 with fp32 PSUM accumulation.  v3 structure:
  - Host pre-tiles every input so each DMA is one fully contiguous block
    (the v2 strided rearrange DMAs left the PE idle ~16us at start).
  - Critical prefix is just K-proj(cc0,tb0), Q-proj(cc0,qb0), V-proj(tb0,
    tt0) after 8 HAM warm-up matmuls; every other projection group is a
    work item interleaved into the attention units' per-kt slack.
  - scores S^T[k,q] via concurrent 64-row tile pairs; exp on ACT out of
    PSUM (scale=0.125, no max subtraction); PV accumulates C^T with a
    ones column -> row 64 = softmax denominator.
  - Late-ready work (norm, outproj) is emitted at background priority so
    the list scheduler cannot splice it between the two concurrent score
    matmuls of a pair when it wakes up.
  - Tail: junk matmuls gated on the last unit's exp tiles keep the HAM
    clock-gate warm through the final drain + output projection.
"""

import numpy as np
import ml_dtypes
import concourse.bass as bass
import concourse.tile as tile
import concourse.mybir as mybir
from concourse import bacc, bass_utils

B, S, D, H = 2, 2048, 1024, 16
DH = 64
HL = 4            # heads per core
CL = HL * DH      # local channels = 256
N_CORES = 8

f32 = mybir.dt.float32
bf16 = mybir.dt.bfloat16
AF = mybir.ActivationFunctionType
BF = ml_dtypes.bfloat16

TB = 4            # token blocks for projections (512 tokens each)
TBS = S // TB     # 512
QB = 4            # query blocks for attention (512 queries each)
QBS = S // QB     # 512
KT_N = S // 128   # 16 key tiles
BG = 10 ** 7      # background priority offset


def build():
    nc = bacc.Bacc("TRN2", debug=False, num_devices=N_CORES)
    # host-pre-tiled inputs: every DMA below is one contiguous block
    qt4 = nc.dram_tensor("qt4", [TB, 128, 8, TBS], bf16, kind="ExternalInput").ap()
    kt4 = nc.dram_tensor("kt4", [TB, 128, 8, TBS], bf16, kind="ExternalInput").ap()
    vt4 = nc.dram_tensor("vt4", [TB, 128, 8, TBS], bf16, kind="ExternalInput").ap()
    wqT = nc.dram_tensor("wqT", [128, 8, CL], bf16, kind="ExternalInput").ap()
    wkT = nc.dram_tensor("wkT", [128, 8, CL], bf16, kind="ExternalInput").ap()
    wvT = nc.dram_tensor("wvT", [128, 8, CL], bf16, kind="ExternalInput").ap()
    woT = nc.dram_tensor("woT", [128, 2, D], bf16, kind="ExternalInput").ap()
    bq = nc.dram_tensor("bq", [128, 2], f32, kind="ExternalInput").ap()
    bk = nc.dram_tensor("bk", [128, 2], f32, kind="ExternalInput").ap()
    vones = nc.dram_tensor("vones", [128, KT_N], bf16, kind="ExternalInput").ap()
    out = nc.dram_tensor("out", [S, D], f32, kind="ExternalOutput").ap()

    with tile.TileContext(nc) as tc:
        with (
            tc.tile_pool(name="wp", bufs=1) as wp,
            tc.tile_pool(name="xp", bufs=6) as xp,
            tc.tile_pool(name="qkv", bufs=1) as qkv,
            tc.tile_pool(name="cp", bufs=1) as cp,
            tc.tile_pool(name="ep", bufs=6) as ep,
            tc.tile_pool(name="rp", bufs=2) as rp,
            tc.tile_pool(name="op", bufs=2) as op,
            tc.tile_pool(name="pp", bufs=2, space="PSUM") as pp,
            tc.tile_pool(name="sp", bufs=2, space="PSUM") as sp,
            tc.tile_pool(name="cps", bufs=1, space="PSUM") as cps,
        ):
            # ---- constants first (no DMA deps) so warm-up can start now ----
            ones_sb = wp.tile([128, 128], bf16)
            nc.vector.memset(ones_sb, 1.0)
            warm_rhs = wp.tile([128, 512], bf16)
            nc.vector.memset(warm_rhs, 0.0)
            # zero rows 0:64,65:128 + denominator row 64 -> K=128 broadcast
            # matmuls that stay in the default (128,128) tiling mode.
            zrow_a = wp.tile([128, 512], bf16)
            nc.vector.memset(zrow_a, 0.0)
            zrow_b = wp.tile([128, 512], bf16)
            nc.vector.memset(zrow_b, 0.0)
            # warm-up matmuls: occupy the PE during the input-DMA wait so the
            # HAM clock gate reaches 8/8 before the projections start.
            for i in range(14):
                wps = pp.tile([128, 512], f32, tag="pp", name="wps")
                nc.tensor.matmul(wps, ones_sb, warm_rhs)

            # ---- weights; DMA order puts the critical-prefix inputs first --
            wk_sb = wp.tile([128, 8, CL], bf16)
            wq_sb = wp.tile([128, 8, CL], bf16)
            wv_sb = wp.tile([128, 8, CL], bf16)
            wo_sb = wp.tile([128, 2, D], bf16)
            bq_sb = wp.tile([128, 2], f32)
            bk_sb = wp.tile([128, 2], f32)

            xk_t = [None] * TB
            xv_t = [None] * TB
            xq_t = [None] * QB

            def dma_x(kind, idx):
                t = xp.tile([128, 8, TBS], bf16, tag="x", name=f"x{kind}{idx}")
                src = {"k": kt4, "v": vt4, "q": qt4}[kind]
                nc.sync.dma_start(t, src[idx])
                return t

            nc.sync.dma_start(wk_sb, wkT)
            nc.sync.dma_start(bk_sb, bk)
            xk_t[0] = dma_x("k", 0)
            nc.sync.dma_start(wq_sb, wqT)
            nc.sync.dma_start(bq_sb, bq)
            xq_t[0] = dma_x("q", 0)
            nc.sync.dma_start(wv_sb, wvT)
            xv_t[0] = dma_x("v", 0)
            nc.sync.dma_start(wo_sb, woT)
            xk_t[1] = dma_x("k", 1)
            xk_t[2] = dma_x("k", 2)
            xk_t[3] = dma_x("k", 3)

            # ---- activations in SBUF ----
            QT = qkv.tile([128, 2, S], bf16)   # [chan, tok]
            KT = qkv.tile([128, 2, S], bf16)
            # V[tok, head-of-4, 65]: col 64 of each head group is the ones
            # column -> PV row 64 is the softmax denominator.
            V = qkv.tile([128, KT_N, 4, 65], bf16)
            for g in range(4):
                nc.sync.dma_start(V[:, :, g, 64], vones)

            # ---- projection group emitters ----
            def k_group(tb, cc):
                t0 = tb * TBS
                ps = pp.tile([128, TBS], f32, tag="pp", name="ps_k")
                for c in range(8):
                    nc.tensor.matmul(ps, wk_sb[:, c, cc * 128:(cc + 1) * 128],
                                     xk_t[tb][:, c], start=(c == 0), stop=(c == 7))
                nc.vector.tensor_scalar_add(KT[:, cc, t0:t0 + TBS], ps,
                                            bk_sb[:, cc:cc + 1])

            def q_group(qb, cc):
                t0 = qb * TBS
                ps = pp.tile([128, TBS], f32, tag="pp", name="ps_q")
                for c in range(8):
                    nc.tensor.matmul(ps, wq_sb[:, c, cc * 128:(cc + 1) * 128],
                                     xq_t[qb][:, c], start=(c == 0), stop=(c == 7))
                nc.vector.tensor_scalar_add(QT[:, cc, t0:t0 + TBS], ps,
                                            bq_sb[:, cc:cc + 1])

            def v_group(tb, tt):
                ps = pp.tile([128, CL], f32, tag="pp", name="ps_v")
                for c in range(8):
                    nc.tensor.matmul(ps, xv_t[tb][:, c, tt * 128:(tt + 1) * 128],
                                     wv_sb[:, c], start=(c == 0), stop=(c == 7))
                T = tb * 4 + tt
                # one fused cast: ps [128,(4,64)] -> V[:,T,:,0:64] (stride 65)
                nc.vector.tensor_copy(
                    V[:, T, :, 0:64],
                    ps.rearrange("p (g c) -> p g c", c=64))

            # ---- prologue projections (minimal; rest interleaves into u0) --
            for cc in range(2):
                k_group(0, cc)
            for cc in range(2):
                q_group(0, cc)
            k_group(1, 1)
            v_group(0, 0)

            # ---- attention ----
            C = cp.tile([128, 2, S], bf16)   # C^T [cat-chan, tok]

            def make_norm(qb, hp, c_a, c_b):
                q0 = qb * QBS

                def norm():
                    s_a = rp.tile([65, QBS], bf16, tag="sa", name="s_a")
                    s_b = rp.tile([65, QBS], bf16, tag="sb", name="s_b")
                    nc.vector.tensor_copy(s_a[64:65, :], c_a[64:65, :])
                    nc.vector.tensor_copy(s_b[64:65, :], c_b[64:65, :])
                    # broadcast sums to 64 partitions, then reciprocal
                    b_ps = pp.tile([64, QBS], f32, tag="pp", name="b_ps")
                    nc.tensor.matmul(b_ps, ones_sb[64:65, 0:64], s_a[64:65, :])
                    r_a = rp.tile([64, QBS], f32, tag="ra", name="r_a")
                    nc.vector.reciprocal_approx_fast(r_a, b_ps)
                    b_ps2 = pp.tile([64, QBS], f32, tag="pp", name="b_ps2")
                    nc.tensor.matmul(b_ps2, ones_sb[64:65, 0:64], s_b[64:65, :])
                    r_b = rp.tile([64, QBS], f32, tag="rb", name="r_b")
                    nc.vector.reciprocal_approx_fast(r_b, b_ps2)
                    nc.vector.tensor_mul(C[0:64, hp, q0:q0 + QBS],
                                         c_a[0:64, :], r_a)
                    nc.vector.tensor_mul(C[64:128, hp, q0:q0 + QBS],
                                         c_b[0:64, :], r_b)
                return norm

            def make_outproj(qb, tt):
                def outproj():
                    tg = qb * QBS + tt * 128
                    o = op.tile([128, D], f32, tag="o")
                    ps0 = pp.tile([128, 512], f32, tag="pp", name="ps0")
                    ps1 = pp.tile([128, 512], f32, tag="pp", name="ps1")
                    for cc in range(2):
                        nc.tensor.matmul(ps0, C[:, cc, tg:tg + 128],
                                         wo_sb[:, cc, 0:512],
                                         start=(cc == 0), stop=(cc == 1))
                        nc.tensor.matmul(ps1, C[:, cc, tg:tg + 128],
                                         wo_sb[:, cc, 512:1024],
                                         start=(cc == 0), stop=(cc == 1))
                    nc.vector.tensor_copy(o[:, 0:512], ps0)
                    nc.vector.tensor_copy(o[:, 512:1024], ps1)
                    nc.sync.dma_start(out[tg:tg + 128, :], o)
                return outproj

            def make_dma(kind, idx, store):
                def dma():
                    store[idx] = dma_x(kind, idx)
                return dma

            def seq(*fns):
                def run():
                    for f in fns:
                        f()
                return run

            def mk(fn, *args):
                return lambda: fn(*args)

            # ---- per-unit interleave schedules ----
            units = [(qb, hp) for qb in range(QB) for hp in range(2)]
            items = {u: [] for u in units}
            # u0 slots kt1..kt15, then leftovers before the PV drain.
            # deadlines: K(tb)g0 before scores kt=4tb; V writer of V[:,j]
            # before kt=j+5 (PV(j) fires there); cc=1 groups before u1.
            items[(0, 0)] = [
                seq(make_dma("v", 1, xv_t), mk(k_group, 1, 0)),   # idx0
                seq(make_dma("v", 2, xv_t), mk(v_group, 0, 1)),   # idx1
                mk(v_group, 0, 2),                                # idx2
                mk(v_group, 0, 3),                                # idx3
                seq(make_dma("v", 3, xv_t), mk(k_group, 2, 0)),   # idx4
                mk(v_group, 1, 0),                                # idx5
                mk(v_group, 1, 1),                                # idx6
                mk(v_group, 1, 2),                                # idx7
                mk(v_group, 1, 3),                                # idx8
                mk(k_group, 3, 0),                                # idx9
                mk(v_group, 2, 0),                                # idx10
                mk(v_group, 2, 1),                                # idx11
                seq(make_dma("q", 1, xq_t), mk(v_group, 2, 2)),   # idx12
                mk(v_group, 2, 3),                                # idx13
                # leftovers (emitted after the kt loop, before the PV drain)
                mk(v_group, 3, 0),
                mk(v_group, 3, 1),
                mk(v_group, 3, 2),
                mk(v_group, 3, 3),
                mk(k_group, 2, 1),
                seq(make_dma("q", 2, xq_t), mk(k_group, 3, 1)),
            ]

            def pv(kt, e, c_a, c_b, hp):
                # C^T accumulation; row 64 = softmax denominators
                nc.tensor.matmul(c_a, V[:, kt, 2 * hp],
                                 e[:, 0:QBS], start=(kt == 0),
                                 stop=(kt == KT_N - 1))
                nc.tensor.matmul(c_b, V[:, kt, 2 * hp + 1],
                                 e[:, QBS:2 * QBS], start=(kt == 0),
                                 stop=(kt == KT_N - 1))

            last_es = []

            def attention_unit(qb, hp, work, keep_es=False):
                q0 = qb * QBS
                c_a = cps.tile([65, QBS], f32, tag="ca", name="c_a")
                c_b = cps.tile([65, QBS], f32, tag="cb", name="c_b")
                pend = []
                it = 0
                for j in range(KT_N // 2):
                    # both kts of the pair back-to-back: one 64-row-mode block
                    pair = []
                    for kt in (2 * j, 2 * j + 1):
                        k0 = kt * 128
                        s_ps = sp.tile([128, 2 * QBS], f32, tag="s")
                        nc.tensor.matmul(s_ps[:, 0:QBS],
                                         KT[0:64, hp, k0:k0 + 128],
                                         QT[0:64, hp, q0:q0 + QBS])
                        nc.tensor.matmul(s_ps[:, QBS:2 * QBS],
                                         KT[64:128, hp, k0:k0 + 128],
                                         QT[64:128, hp, q0:q0 + QBS])
                        pair.append((kt, s_ps))
                    for kt, s_ps in pair:
                        e = ep.tile([128, 2 * QBS], bf16, tag="e")
                        nc.scalar.activation(e, s_ps, AF.Exp, scale=0.125)
                        if keep_es and kt >= 11:
                            last_es.append(e)
                        pend.append((kt, e))
                    if j >= 1:
                        for _ in range(2):
                            if it < len(work):
                                work[it]()
                                it += 1
                    while len(pend) > 4:
                        pv(*pend.pop(0), c_a, c_b, hp)
                while it < len(work):
                    work[it]()
                    it += 1
                for item in pend:
                    pv(*item, c_a, c_b, hp)
                return c_a, c_b

            # schedule: norm(u) runs early in the following unit; outproj(qb)
            # runs in the unit after (qb,1); qproj(qb+1) inside (qb,1).
            prev_cacb = {}
            for ui, (qb, hp) in enumerate(units):
                work = list(items[(qb, hp)])
                if ui >= 1:
                    pu = units[ui - 1]
                    work.insert(0, make_norm(pu[0], pu[1], *prev_cacb[pu]))
                if hp == 1 and qb + 1 < QB:
                    work.insert(1, mk(q_group, qb + 1, 0))
                    work.insert(2, mk(q_group, qb + 1, 1))
                if hp == 0 and qb in (1, 2):
                    opq = qb - 1
                    ext = [make_outproj(opq, t) for t in range(4)]
                    if opq == 0:
                        ext[0] = seq(make_dma("q", 3, xq_t), ext[0])
                    work[1:1] = ext
                if hp == 1 and qb == 3:
                    work[1:1] = [make_outproj(2, t) for t in range(4)]
                prev_cacb[(qb, hp)] = attention_unit(
                    qb, hp, work, keep_es=(ui == len(units) - 1))

            # tail: last norm + last outproj; junk matmuls gated on the last
            # unit's exp tiles keep the HAM clock warm through the drain.
            lu = units[-1]
            make_norm(lu[0], lu[1], *prev_cacb[lu])()
            for t in range(4):
                make_outproj(QB - 1, t)()
            # HAM keep-warm junk through the final exp drain (reads the last
            # unit's e tiles, whose ep buffers are never rotated again)
            for e in last_es:
                for h in range(2):
                    wps = pp.tile([128, 512], f32, tag="pp", name="wps_t")
                    nc.tensor.matmul(wps, ones_sb, e[:, h * 512:h * 512 + 512])

    nc.compile()
    return nc


_CACHE = {}


def _get_nc():
    if "nc" not in _CACHE:
        _CACHE["nc"] = build()
    return _CACHE["nc"]


def make_in_maps(q, k, v, wq, bq, wk, bk, wv, bv, wo, bo):
    def tile4(x):
        # [S, D] activation -> [TB, 128, 8, TBS]; [tb,p,c,t] = x[tb*512+t, c*128+p]
        xT = np.ascontiguousarray(np.asarray(x).T).astype(BF)  # [D, S]
        return np.ascontiguousarray(
            xT.reshape(8, 128, TB, TBS).transpose(2, 1, 0, 3))

    def wtile(w, sl):
        # [D_out slice, D_in] -> [128, 8, CL]; [p,c,n] = w.T[c*128+p, n]
        wT = np.ascontiguousarray(np.asarray(w)[sl, :].T).astype(BF)  # [D, CL]
        return np.ascontiguousarray(wT.reshape(8, 128, CL).transpose(1, 0, 2))

    def wotile(wo_, sl):
        woTc = np.ascontiguousarray(np.asarray(wo_)[:, sl].T).astype(BF)  # [CL, D]
        return np.ascontiguousarray(woTc.reshape(2, 128, D).transpose(1, 0, 2))

    def btile(b, sl):
        return np.ascontiguousarray(
            np.asarray(b)[sl].astype(np.float32).reshape(2, 128).T)

    xt = {}
    for b in range(B):
        xt[("q", b)] = tile4(q[b])
        xt[("k", b)] = tile4(k[b])
        xt[("v", b)] = tile4(v[b])
    in_maps = []
    for core in range(N_CORES):
        b, j = divmod(core, N_CORES // B)
        sl = slice(CL * j, CL * (j + 1))
        in_maps.append({
            "qt4": xt[("q", b)],
            "kt4": xt[("k", b)],
            "vt4": xt[("v", b)],
            "wqT": wtile(wq, sl),
            "wkT": wtile(wk, sl),
            "wvT": wtile(wv, sl),
            "woT": wotile(wo, sl),
            "bq": btile(bq, sl),
            "bk": btile(bk, sl),
            "vones": np.ones((128, KT_N), dtype=BF),
        })
    return in_maps


def combine(results, bv, wo, bo):
    GP = N_CORES // B
    const = (np.asarray(bv, dtype=np.float64) @ np.asarray(wo, dtype=np.float64).T
             + np.asarray(bo, dtype=np.float64)).astype(np.float32)
    out = np.empty((B, S, D), dtype=np.float32)
    for b in range(B):
        acc = results[b * GP]["out"].astype(np.float32).copy()
        for j in range(1, GP):
            acc += results[b * GP + j]["out"]
        out[b] = acc + const[None, :]
    return out


def kernel(q, k, v, wq, bq, wk, bk, wv, bv, wo, bo):
    nc = _get_nc()
    in_maps = make_in_maps(q, k, v, wq, bq, wk, bk, wv, bv, wo, bo)
    res = bass_utils.run_bass_kernel_spmd(nc, in_maps, core_ids=list(range(N_CORES)))
    return combine(res.results, bv, wo, bo)
